# revision 1
# baseline (speedup 1.0000x reference)
"""Kernelized (linear) attention on 8 TRN2 NeuronCores.

Strategy (hardcoded for D=2048, H=16, T=4096, B=4, 8 cores):
  - Data-parallel over history T for the heavy K/V projections: core c gets
    t in [c*512, (c+1)*512). Each core computes, for every head/out-channel o
    and batch b:
        ks[o, b] = sum_t relu(k_hist @ Wk.T + bk)[t, b, o]
        kv[o, b] = sum_t (relu(...)+eps) * (v_hist @ Wv.T + bv)[t, b, o]
    via bf16 matmuls in a transposed layout ([out-feature partitions, (b, t)
    free dim]) so biases are per-partition and reductions run along the free
    dim (fused into the producing ops' accum_out).
  - One tiny AllReduce (64 KB) of the [kv | ks] stats across the 8 cores.
  - Every core then redundantly computes the small final stage (q projection,
    combine, Wo projection) and writes the full [4, 2048] output; the host
    takes core 0's result.
"""

import numpy as np
import ml_dtypes

from concourse import bass, bacc, mybir, tile
from concourse.bass_utils import run_bass_kernel_spmd

BF16 = ml_dtypes.bfloat16

D, H, T, B = 2048, 16, 4096, 4
HD = D // H           # 128
NCORES = 8
TLOC = T // NCORES    # 512 history rows per core
R = TLOC * B          # 2048 projection rows per core
NT = D // 128         # 16 tiles along d (contraction) and o (output)
RC = TLOC             # free-dim chunk = one batch element = 512
EPS = 1e-6
F32 = mybir.dt.float32
BF = mybir.dt.bfloat16
AF = mybir.ActivationFunctionType
OP = mybir.AluOpType


def build_nc():
    nc = bacc.Bacc("TRN2", target_bir_lowering=False, debug=False,
                   enable_asserts=False, num_devices=NCORES)

    def din(name, shape, dt):
        return nc.dram_tensor(name, list(shape), dt, kind="ExternalInput").ap()

    xk_d = din("xk", (D, R), BF)            # k shard, [d, b*512+t]
    xv_d = din("xv", (D, R), BF)            # v shard, [d, b*512+t]
    wk_d = din("wk", (NT, 128, D), BF)      # [ot, p(d%128), t(d//128)*128+o_in]
    wv_d = din("wv", (NT, 128, D), BF)
    wq_d = din("wq", (D, D), BF)            # Wq.T  [d, o]
    wo_d = din("wo", (NT, 128, D), BF)      # [ot, p(o_in), o']
    qt_d = din("qt", (128, NT * B), BF)     # [p, t*4+b] = q[b, t*128+p]
    bk_d = din("bk", (128, NT), F32)        # [p, ot]
    bv_d = din("bv", (128, NT), F32)
    bq_d = din("bq", (128, NT), F32)
    bo_d = din("bo", (B, D), F32)           # bo tiled over 4 partitions
    al_d = din("al", (1, H * B), F32)       # alpha repeated x4 (b-fast)
    eye_d = din("eye", (128, 128), F32)
    onc_d = din("onc", (128, 1), F32)       # ones column
    onr_d = din("onr", (1, 128), F32)       # ones row

    out_d = nc.dram_tensor("out", [B, D], F32, kind="ExternalOutput").ap()

    with tile.TileContext(nc) as tc:
        import contextlib
        with contextlib.ExitStack() as ctx:
            p_xk = ctx.enter_context(tc.tile_pool(name="xk", bufs=NT))
            p_xv = ctx.enter_context(tc.tile_pool(name="xv", bufs=NT))
            p_w = ctx.enter_context(tc.tile_pool(name="w", bufs=2))
            p_ep = ctx.enter_context(tc.tile_pool(name="ep", bufs=2))
            p_pr = ctx.enter_context(tc.tile_pool(name="pr", bufs=2))
            p_c1 = ctx.enter_context(tc.tile_pool(name="c1", bufs=1))
            p_qk = ctx.enter_context(tc.tile_pool(name="qk", bufs=NT))
            p_mm = ctx.enter_context(tc.tile_pool(name="mm", bufs=3, space="PSUM"))
            p_a4 = ctx.enter_context(tc.tile_pool(name="a4", bufs=4, space="PSUM"))
            p_tn = ctx.enter_context(tc.tile_pool(name="tn", bufs=1, space="PSUM"))
            p_dr = ctx.enter_context(tc.tile_pool(name="dr", bufs=1, space="DRAM"))

            # ---- resident loads -------------------------------------------
            xk_t, xv_t = [], []
            for t in range(NT):
                a = p_xk.tile([128, R], BF, tag="xk")
                nc.sync.dma_start(out=a[:], in_=xk_d[t * 128:(t + 1) * 128, :])
                xk_t.append(a)
                b_ = p_xv.tile([128, R], BF, tag="xv")
                nc.sync.dma_start(out=b_[:], in_=xv_d[t * 128:(t + 1) * 128, :])
                xv_t.append(b_)

            bk_s = p_c1.tile([128, NT], F32, tag="bk")
            nc.sync.dma_start(out=bk_s[:], in_=bk_d[:, :])
            bv_s = p_c1.tile([128, NT], F32, tag="bv")
            nc.sync.dma_start(out=bv_s[:], in_=bv_d[:, :])
            bq_s = p_c1.tile([128, NT], F32, tag="bq")
            nc.sync.dma_start(out=bq_s[:], in_=bq_d[:, :])
            bo_s = p_c1.tile([B, D], F32, tag="bo")
            nc.sync.dma_start(out=bo_s[:], in_=bo_d[:, :])
            al_s = p_c1.tile([1, H * B], F32, tag="al")
            nc.sync.dma_start(out=al_s[:], in_=al_d[:, :])
            eye_s = p_c1.tile([128, 128], F32, tag="eye")
            nc.sync.dma_start(out=eye_s[:], in_=eye_d[:, :])
            onc_s = p_c1.tile([128, 1], F32, tag="onc")
            nc.sync.dma_start(out=onc_s[:], in_=onc_d[:, :])
            onr_s = p_c1.tile([1, 128], F32, tag="onr")
            nc.sync.dma_start(out=onr_s[:], in_=onr_d[:, :])
            qt_s = p_c1.tile([128, NT * B], BF, tag="qt")
            nc.sync.dma_start(out=qt_s[:], in_=qt_d[:, :])

            # stats staged in one tile: cols [0:64] kv, [64:128] ks
            stat = p_c1.tile([128, 2 * H * B], F32, tag="stat")

            # ---- main loop: K/V projections + fused stats -----------------
            for ot in range(NT):
                wk_s = p_w.tile([128, D], BF, tag="wk")
                nc.sync.dma_start(out=wk_s[:], in_=wk_d[ot])
                wv_s = p_w.tile([128, D], BF, tag="wv")
                nc.sync.dma_start(out=wv_s[:], in_=wv_d[ot])
                for b in range(B):
                    idx = ot * B + b
                    kp = p_mm.tile([128, RC], F32, tag="mm")
                    for t in range(NT):
                        nc.tensor.matmul(
                            kp[:], wk_s[:, t * 128:(t + 1) * 128],
                            xk_t[t][:, b * RC:(b + 1) * RC],
                            start=(t == 0), stop=(t == NT - 1))
                    vp = p_mm.tile([128, RC], F32, tag="mm")
                    for t in range(NT):
                        nc.tensor.matmul(
                            vp[:], wv_s[:, t * 128:(t + 1) * 128],
                            xv_t[t][:, b * RC:(b + 1) * RC],
                            start=(t == 0), stop=(t == NT - 1))
                    kk = p_ep.tile([128, RC], F32, tag="kk")
                    nc.scalar.activation(
                        kk[:], kp[:], AF.Relu, bias=bk_s[:, ot:ot + 1],
                        scale=1.0, accum_out=stat[:, 64 + idx:64 + idx + 1])
                    vb = p_ep.tile([128, RC], F32, tag="vb")
                    nc.vector.tensor_scalar(
                        vb[:], vp[:], bv_s[:, ot:ot + 1], None, OP.add)
                    pr = p_pr.tile([128, RC], BF, tag="pr")
                    nc.vector.scalar_tensor_tensor(
                        pr[:], kk[:], EPS, vb[:], OP.add, OP.mult,
                        accum_out=stat[:, idx:idx + 1])

            # ---- all-reduce the stats across the 8 cores ------------------
            bin_ = p_dr.tile([128, 2 * H * B], F32, tag="bin")
            bout = p_dr.tile([128, 2 * H * B], F32, tag="bout")
            nc.gpsimd.dma_start(out=bin_[:], in_=stat[:])
            nc.gpsimd.collective_compute(
                "AllReduce", OP.add,
                replica_groups=[list(range(NCORES))],
                ins=[bin_.opt()], outs=[bout.opt()])
            ared = p_c1.tile([128, 2 * H * B], F32, tag="ared")
            nc.gpsimd.dma_start(out=ared[:], in_=bout[:])

            # ---- q projection (redundant on every core) -------------------
            qp = [p_a4.tile([B, 512], F32, tag="a4", name=f"qp{i}")
                  for i in range(4)]
            for t in range(NT):
                wq_s = p_w.tile([128, D], BF, tag="wq")
                nc.sync.dma_start(out=wq_s[:], in_=wq_d[t * 128:(t + 1) * 128, :])
                for oc in range(4):
                    nc.tensor.matmul(
                        qp[oc][:], qt_s[:, t * B:(t + 1) * B],
                        wq_s[:, oc * 512:(oc + 1) * 512],
                        start=(t == 0), stop=(t == NT - 1))
            qraw = p_c1.tile([B, D], F32, tag="big4", name="qraw")
            for oc in range(4):
                nc.vector.tensor_copy(qraw[:, oc * 512:(oc + 1) * 512], qp[oc][:])

            # ---- combine stats --------------------------------------------
            # k_sum per head: sum ks over the 128 partitions of each head
            hs = p_tn.tile([1, H * B], F32, tag="tn")
            nc.tensor.matmul(hs[:], onc_s[:], ared[:, 64:128],
                             start=True, stop=True)
            den = p_c1.tile([1, H * B], F32, tag="den")
            # + EPS*T*HD (the +eps inside k_k summed over T*HD) + outer eps
            nc.vector.tensor_scalar(den[:], hs[:], EPS * T * HD + EPS, None,
                                    OP.add)
            rden = p_c1.tile([1, H * B], F32, tag="rden")
            nc.vector.reciprocal(rden[:], den[:])
            rr = p_c1.tile([1, H * B], F32, tag="rr")
            nc.vector.tensor_tensor(rr[:], rden[:], al_s[:], OP.mult)
            # broadcast rr across partitions, fold into kv
            bcr = p_tn.tile([128, H * B], F32, tag="tn")
            nc.tensor.matmul(bcr[:], onr_s[:], rr[:], start=True, stop=True)
            kvr = p_c1.tile([128, H * B], F32, tag="kvr")
            nc.vector.tensor_tensor(kvr[:], ared[:, 0:64], bcr[:], OP.mult)

            # ---- per-head: transpose q_k, combine, accumulate W_o ---------
            op_ps = [p_a4.tile([B, 512], F32, tag="a4", name=f"op{i}")
                     for i in range(4)]
            for ot in range(NT):
                tp = p_tn.tile([128, B], F32, tag="tn")
                nc.tensor.transpose(tp[:], qraw[:, ot * 128:(ot + 1) * 128],
                                    eye_s[:B, :B])
                qkt = p_qk.tile([128, B], F32, tag="qkt")
                nc.vector.tensor_scalar(qkt[:], tp[:], bq_s[:, ot:ot + 1],
                                        0.0, OP.add, OP.max)
                opre = p_qk.tile([128, B], BF, tag="opre")
                nc.vector.scalar_tensor_tensor(
                    opre[:], qkt[:], EPS, kvr[:, ot * B:(ot + 1) * B],
                    OP.add, OP.mult)
                wo_s = p_w.tile([128, D], BF, tag="wo")
                nc.sync.dma_start(out=wo_s[:], in_=wo_d[ot])
                for oc in range(4):
                    nc.tensor.matmul(
                        op_ps[oc][:], opre[:], wo_s[:, oc * 512:(oc + 1) * 512],
                        start=(ot == 0), stop=(ot == NT - 1))

            outf = p_c1.tile([B, D], F32, tag="big4", name="outf")
            for oc in range(4):
                nc.vector.tensor_tensor(
                    outf[:, oc * 512:(oc + 1) * 512], op_ps[oc][:],
                    bo_s[:, oc * 512:(oc + 1) * 512], OP.add)
            nc.sync.dma_start(out=out_d[:, :], in_=outf[:])

    nc.finalize()  # bacc passes incl. alloc_regs()
    # Strip callback/trap pseudo-instructions (they carry virtual registers
    # that walrus's verifier rejects) — same as MultiCoreSim.run_on_hw_raw.
    from concourse import bass_interp
    nc.m = bass_interp.get_hw_module(nc.m)
    return nc


def prep_inputs(q, k_history, v_history, Wq, bq, Wk, bk, Wv, bv, Wo, bo, alpha):
    """Host-side sharding + layout transforms. Returns in_maps for 8 cores."""
    f32 = np.float32

    def wblocks(W):  # [o,d] -> [ot, p(d%128), (d//128)*128 + o_in] bf16
        a = W.astype(f32).reshape(NT, 128, NT, 128)       # (ot, o_in, t, p)
        return np.ascontiguousarray(a.transpose(0, 3, 2, 1)).astype(BF16) \
                 .reshape(NT, 128, D)

    wk = wblocks(Wk)
    wv = wblocks(Wv)
    wq = np.ascontiguousarray(Wq.astype(f32).T).astype(BF16)        # [d, o]
    wo = np.ascontiguousarray(
        Wo.astype(f32).T.reshape(NT, 128, D)).astype(BF16)          # [ot, p, o']
    qt = np.ascontiguousarray(
        q.astype(f32).T.reshape(NT, 128, B).transpose(1, 0, 2)
    ).astype(BF16).reshape(128, NT * B)                             # [p, t*4+b]
    bk_t = np.ascontiguousarray(bk.astype(f32).reshape(NT, 128).T)
    bv_t = np.ascontiguousarray(bv.astype(f32).reshape(NT, 128).T)
    bq_t = np.ascontiguousarray(bq.astype(f32).reshape(NT, 128).T)
    bo_r = np.ascontiguousarray(np.tile(bo.astype(f32)[None, :], (B, 1)))
    al_r = np.ascontiguousarray(
        np.repeat(alpha.astype(f32), B)[None, :])                   # [1, 64]
    eye = np.eye(128, dtype=f32)
    onc = np.ones((128, 1), f32)
    onr = np.ones((1, 128), f32)

    shared = dict(wk=wk, wv=wv, wq=wq, wo=wo, qt=qt, bk=bk_t, bv=bv_t,
                  bq=bq_t, bo=bo_r, al=al_r, eye=eye, onc=onc, onr=onr)

    in_maps = []
    for c in range(NCORES):
        ks_ = k_history[c * TLOC:(c + 1) * TLOC].astype(f32)   # [512, 4, 2048]
        vs_ = v_history[c * TLOC:(c + 1) * TLOC].astype(f32)
        xk = np.ascontiguousarray(ks_.transpose(2, 1, 0).reshape(D, R)) \
               .astype(BF16)
        xv = np.ascontiguousarray(vs_.transpose(2, 1, 0).reshape(D, R)) \
               .astype(BF16)
        in_maps.append(dict(xk=xk, xv=xv, **shared))
    return in_maps


_CACHE = {}


def kernel(**inputs):
    if "nc" not in _CACHE:
        _CACHE["nc"] = build_nc()
    nc = _CACHE["nc"]
    in_maps = prep_inputs(**{k: np.asarray(v) for k, v in inputs.items()})
    res = run_bass_kernel_spmd(nc, in_maps, core_ids=list(range(NCORES)))
    return np.asarray(res.results[0]["out"], dtype=np.float32)



# revision 3
# speedup vs baseline: 1.0240x; 1.0240x over previous
"""Kernelized (linear) attention on 8 TRN2 NeuronCores — v7 (fp8 DoubleRow matmuls).

vs v1 baseline (same math, same T-data-parallel main loop):
  - Wk/Wv arrive SHARDED (2 of 16 output-tiles per core, 2MB vs 16MB) and
    are AllGather'd on-device; host->device bytes drop ~2.5x overall
    (404MB -> ~162MB across the 8 cores).
  - Stats use ReduceScatter (transposed layout) instead of AllReduce, so
    each core receives exactly its own 2 heads' stats at fixed offsets.
  - Final stage (q proj, combine, Wo) is tensor-parallel: each core only
    computes its own 2 heads with its Wq/Wo shard (1MB+1MB vs 8MB+8MB),
    followed by a 32KB AllReduce of the partial [B,D] output (the
    "row-split W_o + one all-reduce" from the sharding hint).
  - Matmuls use 1024-col bf16 moving operands (half the instructions),
    stationary weight tile shared across both r-chunks.

Layout per core c (hardcoded for D=2048, H=16, T=4096, B=4, 8 cores):
  xk/xv: [D, R=2048] bf16, columns r = b*512 + t_local, t in [c*512,(c+1)*512)
  wkv:   [512, D] bf16 rows = [kv(2), otl(2), p(128)]; block ot = 2c+otl in
         wblocks layout [p = d%128, (d//128)*128 + o_in]
  wq:    [2, 128, D] bf16 wblocks layout for ot = 2c+otl
  wo:    [2, 128, D] bf16 = Wo.T[ot block rows, :] for ot = 2c+otl
  qt:    [128, NT*B] bf16, [p = d%128, (d//128)*B + b]
  Stats column index (before transpose+ReduceScatter):
    kv(ot,b) -> (ot//2)*16 + (ot%2)*4 + b ;  ks(ot,b) -> same + 8
  so RS chunk c = rows [16c, 16c+16) of the transposed [128,128] stats.
"""

import numpy as np
import ml_dtypes

from concourse import bass, bacc, mybir, tile
from concourse.bass_utils import run_bass_kernel_spmd

BF16 = ml_dtypes.bfloat16

D, H, T, B = 2048, 16, 4096, 4
HD = D // H           # 128
NCORES = 8
TLOC = T // NCORES    # 512 history rows per core
R = TLOC * B          # 2048 projection rows per core
NT = D // 128         # 16 tiles along d (contraction) and o (output)
OTC = NT // NCORES    # 2 output tiles (heads) owned per core
EPS = 1e-6
F32 = mybir.dt.float32
BF = mybir.dt.bfloat16
F8 = mybir.dt.float8e4
WSC = 64.0    # host pre-scale on Wk/Wv/Wq/Wo so fp8 stays in normal range
OSC = 8192.0  # on-chip pre-scale on opre (tiny values) before fp8 cast
AF = mybir.ActivationFunctionType
OP = mybir.AluOpType


def build_nc():
    nc = bacc.Bacc("TRN2", target_bir_lowering=False, debug=False,
                   enable_asserts=False, num_devices=NCORES)

    def din(name, shape, dt):
        return nc.dram_tensor(name, list(shape), dt, kind="ExternalInput").ap()

    xk_d = din("xk", (D, R), F8)
    xv_d = din("xv", (D, R), F8)
    wkv_d = din("wkv", (2 * OTC * 128, D), F8)
    wq_d = din("wq", (OTC, 128, D), F8)
    wo_d = din("wo", (OTC, 128, D), F8)
    qt_d = din("qt", (128, NT * B), F8)
    bk_d = din("bk", (128, NT), F32)
    bv8_d = din("bv8", (128, OTC * B), F32)
    bq_d = din("bq", (128, OTC), F32)
    bo8_d = din("bo8", (B, D), F32)       # bo / NCORES, tiled over 4 rows
    al_d = din("al", (1, OTC * B), F32)
    eye_d = din("eye", (128, 128), F32)
    onc_d = din("onc", (128, 1), F32)
    onr_d = din("onr", (1, 128), F32)

    # [128, 64] row-major == [B, D] row-major flat; reshaped host-side
    out_d = nc.dram_tensor("out", [128, (B * D) // 128], F32,
                           kind="ExternalOutput").ap()

    with tile.TileContext(nc) as tc:
        import contextlib
        with contextlib.ExitStack() as ctx:
            p_xk = ctx.enter_context(tc.tile_pool(name="xk", bufs=NT))
            p_xv = ctx.enter_context(tc.tile_pool(name="xv", bufs=NT))
            p_w = ctx.enter_context(tc.tile_pool(name="w", bufs=2))
            p_wo = ctx.enter_context(tc.tile_pool(name="wo", bufs=1))
            p_ep = ctx.enter_context(tc.tile_pool(name="ep", bufs=3))
            p_pr = ctx.enter_context(tc.tile_pool(name="pr", bufs=2))
            p_c1 = ctx.enter_context(tc.tile_pool(name="c1", bufs=1))
            p_qk = ctx.enter_context(tc.tile_pool(name="qk", bufs=2))
            p_st = ctx.enter_context(tc.tile_pool(name="st", bufs=2))
            p_dr = ctx.enter_context(tc.tile_pool(name="dr", bufs=1, space="DRAM"))

            # ---- stage + AllGather the Wk/Wv shards ------------------------
            # (bounce via SBUF: collectives need internal DRAM in/out tiles)
            wg_in = p_dr.tile([2 * OTC * 128, D], F8, tag="wgin")
            for i in range(2 * OTC):
                wtmp = p_st.tile([128, D], F8, tag="wst")
                nc.sync.dma_start(out=wtmp[:],
                                  in_=wkv_d[i * 128:(i + 1) * 128, :])
                nc.gpsimd.dma_start(out=wg_in[i * 128:(i + 1) * 128, :],
                                    in_=wtmp[:])
            wg_out = p_dr.tile([NCORES * 2 * OTC * 128, D], F8, tag="wgout",
                                   addr_space="Shared")
            nc.gpsimd.collective_compute(
                "AllGather", OP.bypass,
                replica_groups=[list(range(NCORES))],
                ins=[wg_in.opt()], outs=[wg_out.opt()])

            # ---- resident loads -------------------------------------------
            xk_t, xv_t = [], []
            for tt in range(NT // 2):
                a = p_xk.tile([128, 2, R], F8, tag="xk")
                for s in range(2):
                    r0 = (2 * tt + s) * 128
                    nc.sync.dma_start(out=a[:, s], in_=xk_d[r0:r0 + 128, :])
                xk_t.append(a)
                b_ = p_xv.tile([128, 2, R], F8, tag="xv")
                for s in range(2):
                    r0 = (2 * tt + s) * 128
                    nc.sync.dma_start(out=b_[:, s], in_=xv_d[r0:r0 + 128, :])
                xv_t.append(b_)

            bk_s = p_c1.tile([128, NT], F32, tag="bk")
            nc.sync.dma_start(out=bk_s[:], in_=bk_d[:, :])
            bv8_s = p_c1.tile([128, OTC * B], F32, tag="bv8")
            nc.sync.dma_start(out=bv8_s[:], in_=bv8_d[:, :])
            bq_s = p_c1.tile([128, OTC], F32, tag="bq")
            nc.sync.dma_start(out=bq_s[:], in_=bq_d[:, :])
            bo8_s = p_c1.tile([B, D], F32, tag="bo8")
            nc.sync.dma_start(out=bo8_s[:], in_=bo8_d[:, :])
            al_s = p_c1.tile([1, OTC * B], F32, tag="al")
            nc.sync.dma_start(out=al_s[:], in_=al_d[:, :])
            eye_s = p_c1.tile([128, 128], F32, tag="eye")
            nc.sync.dma_start(out=eye_s[:], in_=eye_d[:, :])
            onc_s = p_c1.tile([128, 1], F32, tag="onc")
            nc.sync.dma_start(out=onc_s[:], in_=onc_d[:, :])
            onr_s = p_c1.tile([1, 128], F32, tag="onr")
            nc.sync.dma_start(out=onr_s[:], in_=onr_d[:, :])
            qt_s = p_c1.tile([128, NT * B], F8, tag="qt")
            nc.sync.dma_start(out=qt_s[:], in_=qt_d[:, :])
            wq_s = []
            for ol in range(OTC):
                w = p_c1.tile([128, D], F8, tag=f"wq{ol}")
                nc.sync.dma_start(out=w[:], in_=wq_d[ol])
                wq_s.append(w)

            # stats, transposed-RS column order (see module docstring)
            stat = p_c1.tile([128, 2 * H * B], F32, tag="stat")

            def kv_col(ot, b):
                return (ot // OTC) * 16 + (ot % OTC) * 4 + b

            # ---- main loop: K/V projections + fused stats -----------------
            with tc.tile_pool(name="mmk", bufs=3, space="PSUM") as p_mk, \
                 tc.tile_pool(name="mmv", bufs=3, space="PSUM") as p_mv:
                NTT = NT // 2
                DR = mybir.MatmulPerfMode.DoubleRow
                for ot in range(NT):
                    csrc, otl = divmod(ot, OTC)
                    base = csrc * (2 * OTC * 128) + otl * 128
                    wk_s = p_w.tile([128, NTT, 2, 128], F8, tag="wk")
                    nc.sync.dma_start(
                        out=wk_s[:], in_=wg_out[base:base + 128, :])
                    wv_s = p_w.tile([128, NTT, 2, 128], F8, tag="wv")
                    nc.sync.dma_start(
                        out=wv_s[:],
                        in_=wg_out[base + OTC * 128:base + (OTC + 1) * 128, :])
                    for b in range(B):           # 512-col (per-batch) chunks
                        c0 = b * 512
                        ck = kv_col(ot, b)
                        kp = p_mk.tile([128, 512], F32, tag="mmk")
                        for tt in range(NTT):
                            nc.tensor.matmul(
                                kp[:], wk_s[:, tt],
                                xk_t[tt][:, :, c0:c0 + 512],
                                start=(tt == 0), stop=(tt == NTT - 1),
                                perf_mode=DR)
                        vp = p_mv.tile([128, 512], F32, tag="mmv")
                        for tt in range(NTT):
                            nc.tensor.matmul(
                                vp[:], wv_s[:, tt],
                                xv_t[tt][:, :, c0:c0 + 512],
                                start=(tt == 0), stop=(tt == NTT - 1),
                                perf_mode=DR)
                        kk = p_ep.tile([128, 512], F32, tag="kk")
                        nc.scalar.activation(
                            kk[:], kp[:], AF.Relu,
                            bias=bk_s[:, ot:ot + 1], scale=1.0 / WSC,
                            accum_out=stat[:, 8 + ck:8 + ck + 1])
                        pr = p_pr.tile([128, 512], BF, tag="pr")
                        nc.vector.scalar_tensor_tensor(
                            pr[:], kk[:], EPS, vp[:], OP.add, OP.mult,
                            accum_out=stat[:, ck:ck + 1])

            # NOTE on stat columns: kv_col gives the within-chunk offsets
            # 0..7 (kv) and 8..15 (ks) for chunk csrc; chunk base is 16*csrc.
            # kv at 16*csrc + otl*4 + b  = kv_col(ot,b)
            # ks at 16*csrc + 8 + otl*4 + b = kv_col(ot,b) + 8
            # (the expressions above index stat accordingly)

            with tc.tile_pool(name="fin", bufs=2, space="PSUM") as p_fin, \
                 tc.tile_pool(name="tn", bufs=2, space="PSUM") as p_tn, \
                 tc.tile_pool(name="opp", bufs=1, space="PSUM") as p_op:
                # ---- transpose stats, ReduceScatter ------------------------
                stT_ps = p_tn.tile([128, 128], F32, tag="tn", name="stTp")
                nc.tensor.transpose(stT_ps[:], stat[:], eye_s[:])
                stT = p_st.tile([128, 128], F32, tag="stT")
                nc.vector.tensor_copy(stT[:], stT_ps[:])
                rs_in = p_dr.tile([128, 128], F32, tag="rsin")
                nc.gpsimd.dma_start(out=rs_in[:], in_=stT[:])
                rs_out = p_dr.tile([16, 128], F32, tag="rsout")
                nc.gpsimd.collective_compute(
                    "ReduceScatter", OP.add,
                    replica_groups=[list(range(NCORES))],
                    ins=[rs_in.opt()], outs=[rs_out.opt()])

                # ---- q projection for own 2 output tiles (overlaps RS) ----
                qps = []
                for ol in range(OTC):
                    qp = p_fin.tile([128, B], F32, tag="fin", name=f"qp{ol}")
                    for t in range(NT):
                        nc.tensor.matmul(
                            qp[:], wq_s[ol][:, t * 128:(t + 1) * 128],
                            qt_s[:, t * B:(t + 1) * B],
                            start=(t == 0), stop=(t == NT - 1))
                    qps.append(qp)

                # own stats back to [128 channels, 16]: cols 0:8 kv, 8:16 ks
                rsb = p_st.tile([16, 128], F32, tag="rsb")
                nc.gpsimd.dma_start(out=rsb[:], in_=rs_out[:])
                own_ps = p_tn.tile([128, 16], F32, tag="tn", name="ownp")
                nc.tensor.transpose(own_ps[:], rsb[:], eye_s[:16, :16])
                own = p_st.tile([128, 16], F32, tag="own")
                nc.vector.tensor_copy(own[:], own_ps[:])
                kse = p_st.tile([128, OTC * B], F32, tag="kse")
                nc.vector.tensor_scalar(kse[:], own[:, 8:16], T * EPS, None,
                                        OP.add)
                kvb = p_st.tile([128, OTC * B], F32, tag="kvb")
                nc.vector.tensor_tensor(kvb[:], kse[:], bv8_s[:], OP.mult)
                kvc = p_st.tile([128, OTC * B], F32, tag="kvc")
                nc.vector.scalar_tensor_tensor(
                    kvc[:], own[:, 0:8], 1.0 / WSC, kvb[:], OP.mult, OP.add)

                # ---- combine stats for own 2 heads -------------------------
                hs = p_tn.tile([1, OTC * B], F32, tag="tn", name="hs")
                nc.tensor.matmul(hs[:], onc_s[:], own[:, 8:16],
                                 start=True, stop=True)
                den = p_c1.tile([1, OTC * B], F32, tag="den")
                nc.vector.tensor_scalar(den[:], hs[:], EPS * T * HD + EPS,
                                        None, OP.add)
                rden = p_c1.tile([1, OTC * B], F32, tag="rden")
                nc.vector.reciprocal(rden[:], den[:])
                rr = p_c1.tile([1, OTC * B], F32, tag="rr")
                nc.vector.tensor_tensor(rr[:], rden[:], al_s[:], OP.mult)
                # scale opre into fp8 normal range; undone after Wo matmul
                nc.vector.tensor_scalar(rr[:], rr[:], OSC, None, OP.mult)
                bcr = p_tn.tile([128, OTC * B], F32, tag="tn", name="bcr")
                nc.tensor.matmul(bcr[:], onr_s[:], rr[:], start=True,
                                 stop=True)
                kvr = p_c1.tile([128, OTC * B], F32, tag="kvr")
                nc.vector.tensor_tensor(kvr[:], kvc[:], bcr[:], OP.mult)

                # ---- own-head epilogue + row-split Wo ----------------------
                op_ps = p_op.tile([B, D], F32, tag="opp")
                for ol in range(OTC):
                    qkt = p_qk.tile([128, B], F32, tag="qkt")
                    nc.scalar.activation(qkt[:], qps[ol][:], AF.Relu,
                                         bias=bq_s[:, ol:ol + 1],
                                         scale=1.0 / WSC)
                    opre = p_qk.tile([128, B], F8, tag="opre")
                    nc.vector.scalar_tensor_tensor(
                        opre[:], qkt[:], EPS,
                        kvr[:, ol * B:(ol + 1) * B], OP.add, OP.mult)
                    wo_s = p_wo.tile([128, D], F8, tag="wo")
                    nc.sync.dma_start(out=wo_s[:], in_=wo_d[ol])
                    for hh in range(4):
                        nc.tensor.matmul(
                            op_ps[:, hh * 512:(hh + 1) * 512], opre[:],
                            wo_s[:, hh * 512:(hh + 1) * 512],
                            start=(ol == 0), stop=(ol == OTC - 1))

                # un-scale (opre*OSC @ wo*WSC), fold in bo/8 per core
                opart = p_c1.tile([B, D], F32, tag="opart")
                nc.vector.scalar_tensor_tensor(
                    opart[:], op_ps[:], 1.0 / (OSC * WSC), bo8_s[:],
                    OP.mult, OP.add)

            # ---- all-reduce partial outputs -------------------------------
            or_in = p_dr.tile([B, D], F32, tag="orin")
            or_out = p_dr.tile([128, (B * D) // 128], F32, tag="orout",
                               addr_space="Shared")
            nc.gpsimd.dma_start(out=or_in[:], in_=opart[:])
            nc.gpsimd.collective_compute(
                "AllReduce", OP.add,
                replica_groups=[list(range(NCORES))],
                ins=[or_in.opt()], outs=[or_out.opt()])
            osum = p_c1.tile([128, (B * D) // 128], F32, tag="osum")
            nc.gpsimd.dma_start(out=osum[:], in_=or_out[:])
            nc.sync.dma_start(out=out_d[:, :], in_=osum[:])

    nc.finalize()
    from concourse import bass_interp
    nc.m = bass_interp.get_hw_module(nc.m)
    return nc


def prep_inputs(q, k_history, v_history, Wq, bq, Wk, bk, Wv, bv, Wo, bo, alpha):
    """Host-side sharding + layout transforms. Returns in_maps for 8 cores."""
    f32 = np.float32

    def wblocks(W):  # [o,d] -> [ot, p(d%128), (d//128)*128 + o_in] f32
        a = W.astype(f32).reshape(NT, 128, NT, 128)       # (ot, o_in, t, p)
        return np.ascontiguousarray(a.transpose(0, 3, 2, 1)) \
                 .reshape(NT, 128, D)

    wkb = wblocks(Wk)
    wvb = wblocks(Wv)
    F8H = ml_dtypes.float8_e4m3
    wqb = wblocks(Wq)
    wob = np.ascontiguousarray(
        Wo.astype(f32).T.reshape(NT, 128, D))               # [ot, p(o_in), o']
    qt = np.ascontiguousarray(
        q.astype(f32).T.reshape(NT, 128, B).transpose(1, 0, 2)
    ).reshape(128, NT * B)                                  # [p, t*4+b]
    bk_t = np.ascontiguousarray(bk.astype(f32).reshape(NT, 128).T)
    bv_t = np.ascontiguousarray(bv.astype(f32).reshape(NT, 128).T)  # [128, NT]
    bq_t = np.ascontiguousarray(bq.astype(f32).reshape(NT, 128).T)
    bo8_r = np.ascontiguousarray(
        np.tile(bo.astype(f32)[None, :] / NCORES, (B, 1)))
    eye = np.eye(128, dtype=f32)
    onc = np.ones((128, 1), f32)
    onr = np.ones((1, 128), f32)
    alpha = np.asarray(alpha, f32)

    qt = qt.astype(F8H)
    shared = dict(qt=qt, bk=bk_t, bo8=bo8_r, eye=eye, onc=onc,
                  onr=onr)

    # cast histories to fp8 once, then per-core strided transpose (1-byte)
    kb = np.asarray(k_history, f32).astype(F8H)             # [T, B, D]
    vb = np.asarray(v_history, f32).astype(F8H)

    in_maps = []
    for c in range(NCORES):
        xk = np.ascontiguousarray(
            kb[c * TLOC:(c + 1) * TLOC].transpose(2, 1, 0)).reshape(D, R)
        xv = np.ascontiguousarray(
            vb[c * TLOC:(c + 1) * TLOC].transpose(2, 1, 0)).reshape(D, R)
        wkv = np.concatenate([wkb[OTC * c:OTC * (c + 1)].reshape(OTC * 128, D),
                              wvb[OTC * c:OTC * (c + 1)].reshape(OTC * 128, D)],
                             axis=0) * WSC
        wkv = wkv.astype(F8H)
        in_maps.append(dict(
            xk=xk, xv=xv, wkv=np.ascontiguousarray(wkv),
            wq=(np.ascontiguousarray(wqb[OTC * c:OTC * (c + 1)])
                .astype(f32) * WSC).astype(F8H),
            wo=(np.ascontiguousarray(wob[OTC * c:OTC * (c + 1)])
                * WSC).astype(F8H),
            bq=np.ascontiguousarray(bq_t[:, OTC * c:OTC * (c + 1)]),
            al=np.ascontiguousarray(
                np.repeat(alpha[OTC * c:OTC * (c + 1)], B)[None, :]),
            bv8=np.ascontiguousarray(
                np.repeat(bv_t[:, OTC * c:OTC * (c + 1)], B, axis=1)),
            **shared))
    return in_maps


_CACHE = {}


def kernel(**inputs):
    if "nc" not in _CACHE:
        _CACHE["nc"] = build_nc()
    nc = _CACHE["nc"]
    in_maps = prep_inputs(**{k: np.asarray(v) for k, v in inputs.items()})
    res = run_bass_kernel_spmd(nc, in_maps, core_ids=list(range(NCORES)))
    return np.asarray(res.results[0]["out"], dtype=np.float32).reshape(B, D)


# revision 4
# speedup vs baseline: 1.0471x; 1.0225x over previous
"""Kernelized (linear) attention on 8 TRN2 NeuronCores — v9 (fp8 DoubleRow, fused tail).

vs v1 baseline (same math, same T-data-parallel main loop):
  - Wk/Wv arrive SHARDED (2 of 16 output-tiles per core, 2MB vs 16MB) and
    are AllGather'd on-device; host->device bytes drop ~2.5x overall
    (404MB -> ~162MB across the 8 cores).
  - Stats use ReduceScatter (transposed layout) instead of AllReduce, so
    each core receives exactly its own 2 heads' stats at fixed offsets.
  - Final stage (q proj, combine, Wo) is tensor-parallel: each core only
    computes its own 2 heads with its Wq/Wo shard (1MB+1MB vs 8MB+8MB),
    followed by a 32KB AllReduce of the partial [B,D] output (the
    "row-split W_o + one all-reduce" from the sharding hint).
  - Matmuls use 1024-col bf16 moving operands (half the instructions),
    stationary weight tile shared across both r-chunks.

Layout per core c (hardcoded for D=2048, H=16, T=4096, B=4, 8 cores):
  xk/xv: [D, R=2048] bf16, columns r = b*512 + t_local, t in [c*512,(c+1)*512)
  wkv:   [512, D] bf16 rows = [kv(2), otl(2), p(128)]; block ot = 2c+otl in
         wblocks layout [p = d%128, (d//128)*128 + o_in]
  wq:    [2, 128, D] bf16 wblocks layout for ot = 2c+otl
  wo:    [2, 128, D] bf16 = Wo.T[ot block rows, :] for ot = 2c+otl
  qt:    [128, NT*B] bf16, [p = d%128, (d//128)*B + b]
  Stats column index (before transpose+ReduceScatter):
    kv(ot,b) -> (ot//2)*16 + (ot%2)*4 + b ;  ks(ot,b) -> same + 8
  so RS chunk c = rows [16c, 16c+16) of the transposed [128,128] stats.
"""

import numpy as np
import ml_dtypes

from concourse import bass, bacc, mybir, tile
from concourse.bass_utils import run_bass_kernel_spmd

BF16 = ml_dtypes.bfloat16

D, H, T, B = 2048, 16, 4096, 4
HD = D // H           # 128
NCORES = 8
TLOC = T // NCORES    # 512 history rows per core
R = TLOC * B          # 2048 projection rows per core
NT = D // 128         # 16 tiles along d (contraction) and o (output)
OTC = NT // NCORES    # 2 output tiles (heads) owned per core
EPS = 1e-6
F32 = mybir.dt.float32
BF = mybir.dt.bfloat16
F8 = mybir.dt.float8e4
WSC = 64.0    # host pre-scale on Wk/Wv/Wq/Wo so fp8 stays in normal range
OSC = 8192.0  # on-chip pre-scale on opre (tiny values) before fp8 cast
AF = mybir.ActivationFunctionType
OP = mybir.AluOpType


def build_nc():
    nc = bacc.Bacc("TRN2", target_bir_lowering=False, debug=False,
                   enable_asserts=False, num_devices=NCORES)

    def din(name, shape, dt):
        return nc.dram_tensor(name, list(shape), dt, kind="ExternalInput").ap()

    xk_d = din("xk", (D, R), F8)
    xv_d = din("xv", (D, R), F8)
    wkv_d = din("wkv", (2 * OTC * 128, D), F8)
    wq_d = din("wq", (OTC, 128, D), F8)
    wo_d = din("wo", (OTC, 128, D), F8)
    qt_d = din("qt", (128, NT * B), F8)
    bk_d = din("bk", (128, NT), F32)
    bv8_d = din("bv8", (128, OTC * B), F32)
    bq_d = din("bq", (128, OTC), F32)
    bo8_d = din("bo8", (B, D), F32)       # bo / NCORES, tiled over 4 rows
    al_d = din("al", (1, OTC * B), F32)
    eye_d = din("eye", (128, 128), F32)
    onc_d = din("onc", (128, 1), F32)
    onr_d = din("onr", (1, 128), F32)

    # [128, 64] row-major == [B, D] row-major flat; reshaped host-side
    out_d = nc.dram_tensor("out", [128, (B * D) // 128], F32,
                           kind="ExternalOutput").ap()

    with tile.TileContext(nc) as tc:
        import contextlib
        with contextlib.ExitStack() as ctx:
            p_xk = ctx.enter_context(tc.tile_pool(name="xk", bufs=NT))
            p_xv = ctx.enter_context(tc.tile_pool(name="xv", bufs=NT))
            p_w = ctx.enter_context(tc.tile_pool(name="w", bufs=2))
            p_ep = ctx.enter_context(tc.tile_pool(name="ep", bufs=3))
            p_pr = ctx.enter_context(tc.tile_pool(name="pr", bufs=2))
            p_c1 = ctx.enter_context(tc.tile_pool(name="c1", bufs=1))
            p_qk = ctx.enter_context(tc.tile_pool(name="qk", bufs=2))
            p_st = ctx.enter_context(tc.tile_pool(name="st", bufs=2))
            p_dr = ctx.enter_context(tc.tile_pool(name="dr", bufs=1, space="DRAM"))

            # ---- stage + AllGather the Wk/Wv shards ------------------------
            # (bounce via SBUF: collectives need internal DRAM in/out tiles)
            wg_in = p_dr.tile([2 * OTC * 128, D], F8, tag="wgin")
            for i in range(2 * OTC):
                wtmp = p_st.tile([128, D], F8, tag="wst")
                nc.sync.dma_start(out=wtmp[:],
                                  in_=wkv_d[i * 128:(i + 1) * 128, :])
                nc.gpsimd.dma_start(out=wg_in[i * 128:(i + 1) * 128, :],
                                    in_=wtmp[:])
            wg_out = p_dr.tile([NCORES * 2 * OTC * 128, D], F8, tag="wgout",
                                   addr_space="Shared")
            nc.gpsimd.collective_compute(
                "AllGather", OP.bypass,
                replica_groups=[list(range(NCORES))],
                ins=[wg_in.opt()], outs=[wg_out.opt()])

            # ---- resident loads -------------------------------------------
            xk_t, xv_t = [], []
            for tt in range(NT // 2):
                a = p_xk.tile([128, 2, R], F8, tag="xk")
                for s in range(2):
                    r0 = (2 * tt + s) * 128
                    nc.sync.dma_start(out=a[:, s], in_=xk_d[r0:r0 + 128, :])
                xk_t.append(a)
                b_ = p_xv.tile([128, 2, R], F8, tag="xv")
                for s in range(2):
                    r0 = (2 * tt + s) * 128
                    nc.sync.dma_start(out=b_[:, s], in_=xv_d[r0:r0 + 128, :])
                xv_t.append(b_)

            bk_s = p_c1.tile([128, NT], F32, tag="bk")
            nc.sync.dma_start(out=bk_s[:], in_=bk_d[:, :])
            bv8_s = p_c1.tile([128, OTC * B], F32, tag="bv8")
            nc.sync.dma_start(out=bv8_s[:], in_=bv8_d[:, :])
            bq_s = p_c1.tile([128, OTC], F32, tag="bq")
            nc.sync.dma_start(out=bq_s[:], in_=bq_d[:, :])
            bo8_s = p_c1.tile([B, D], F32, tag="bo8")
            nc.sync.dma_start(out=bo8_s[:], in_=bo8_d[:, :])
            al_s = p_c1.tile([1, OTC * B], F32, tag="al")
            nc.sync.dma_start(out=al_s[:], in_=al_d[:, :])
            eye_s = p_c1.tile([128, 128], F32, tag="eye")
            nc.sync.dma_start(out=eye_s[:], in_=eye_d[:, :])
            onc_s = p_c1.tile([128, 1], F32, tag="onc")
            nc.sync.dma_start(out=onc_s[:], in_=onc_d[:, :])
            onr_s = p_c1.tile([1, 128], F32, tag="onr")
            nc.sync.dma_start(out=onr_s[:], in_=onr_d[:, :])
            qt_s = p_c1.tile([128, NT * B], F8, tag="qt")
            nc.sync.dma_start(out=qt_s[:], in_=qt_d[:, :])
            wq_s, wo_sl = [], []
            for ol in range(OTC):
                w = p_c1.tile([128, D], F8, tag=f"wq{ol}")
                nc.sync.dma_start(out=w[:], in_=wq_d[ol])
                wq_s.append(w)
                wo_ = p_c1.tile([128, D], F8, tag=f"wo{ol}")
                nc.sync.dma_start(out=wo_[:], in_=wo_d[ol])
                wo_sl.append(wo_)

            # stats, transposed-RS column order (see module docstring)
            stat = p_c1.tile([128, 2 * H * B], F32, tag="stat")

            def kv_col(ot, b):
                return (ot // OTC) * 16 + (ot % OTC) * 4 + b

            # ---- main loop: K/V projections + fused stats -----------------
            with tc.tile_pool(name="mmk", bufs=3, space="PSUM") as p_mk, \
                 tc.tile_pool(name="mmv", bufs=3, space="PSUM") as p_mv:
                NTT = NT // 2
                DR = mybir.MatmulPerfMode.DoubleRow
                for ot in range(NT):
                    csrc, otl = divmod(ot, OTC)
                    base = csrc * (2 * OTC * 128) + otl * 128
                    wk_s = p_w.tile([128, NTT, 2, 128], F8, tag="wk")
                    nc.sync.dma_start(
                        out=wk_s[:], in_=wg_out[base:base + 128, :])
                    wv_s = p_w.tile([128, NTT, 2, 128], F8, tag="wv")
                    nc.sync.dma_start(
                        out=wv_s[:],
                        in_=wg_out[base + OTC * 128:base + (OTC + 1) * 128, :])
                    for b in range(B):           # 512-col (per-batch) chunks
                        c0 = b * 512
                        ck = kv_col(ot, b)
                        kp = p_mk.tile([128, 512], F32, tag="mmk")
                        for tt in range(NTT):
                            nc.tensor.matmul(
                                kp[:], wk_s[:, tt],
                                xk_t[tt][:, :, c0:c0 + 512],
                                start=(tt == 0), stop=(tt == NTT - 1),
                                perf_mode=DR)
                        vp = p_mv.tile([128, 512], F32, tag="mmv")
                        for tt in range(NTT):
                            nc.tensor.matmul(
                                vp[:], wv_s[:, tt],
                                xv_t[tt][:, :, c0:c0 + 512],
                                start=(tt == 0), stop=(tt == NTT - 1),
                                perf_mode=DR)
                        kk = p_ep.tile([128, 512], F32, tag="kk")
                        nc.scalar.activation(
                            kk[:], kp[:], AF.Relu,
                            bias=bk_s[:, ot:ot + 1], scale=1.0 / WSC,
                            accum_out=stat[:, 8 + ck:8 + ck + 1])
                        pr = p_pr.tile([128, 512], BF, tag="pr")
                        nc.vector.scalar_tensor_tensor(
                            pr[:], kk[:], EPS, vp[:], OP.add, OP.mult,
                            accum_out=stat[:, ck:ck + 1])

            # NOTE on stat columns: kv_col gives the within-chunk offsets
            # 0..7 (kv) and 8..15 (ks) for chunk csrc; chunk base is 16*csrc.
            # kv at 16*csrc + otl*4 + b  = kv_col(ot,b)
            # ks at 16*csrc + 8 + otl*4 + b = kv_col(ot,b) + 8
            # (the expressions above index stat accordingly)

            with tc.tile_pool(name="fin", bufs=2, space="PSUM") as p_fin, \
                 tc.tile_pool(name="tn", bufs=2, space="PSUM") as p_tn, \
                 tc.tile_pool(name="opp", bufs=1, space="PSUM") as p_op:
                # ---- transpose stats, ReduceScatter ------------------------
                stT_ps = p_tn.tile([128, 128], F32, tag="tn", name="stTp")
                nc.tensor.transpose(stT_ps[:], stat[:], eye_s[:])
                stT = p_st.tile([128, 128], F32, tag="stT")
                nc.vector.tensor_copy(stT[:], stT_ps[:])
                rs_in = p_dr.tile([128, 128], F32, tag="rsin")
                nc.gpsimd.dma_start(out=rs_in[:], in_=stT[:])
                rs_out = p_dr.tile([16, 128], F32, tag="rsout")
                nc.gpsimd.collective_compute(
                    "ReduceScatter", OP.add,
                    replica_groups=[list(range(NCORES))],
                    ins=[rs_in.opt()], outs=[rs_out.opt()])

                # ---- q projection for own 2 output tiles (overlaps RS) ----
                qps = []
                for ol in range(OTC):
                    qp = p_fin.tile([128, B], F32, tag="fin", name=f"qp{ol}")
                    for t in range(NT):
                        nc.tensor.matmul(
                            qp[:], wq_s[ol][:, t * 128:(t + 1) * 128],
                            qt_s[:, t * B:(t + 1) * B],
                            start=(t == 0), stop=(t == NT - 1))
                    qps.append(qp)

                # own stats back to [128 channels, 16]: cols 0:8 kv, 8:16 ks
                rsb = p_st.tile([16, 128], F32, tag="rsb")
                nc.gpsimd.dma_start(out=rsb[:], in_=rs_out[:])
                own_ps = p_tn.tile([128, 16], F32, tag="tn", name="ownp")
                nc.tensor.transpose(own_ps[:], rsb[:], eye_s[:16, :16])
                own = p_st.tile([128, 16], F32, tag="own")
                nc.vector.tensor_copy(own[:], own_ps[:])
                kvb = p_st.tile([128, OTC * B], F32, tag="kvb")
                nc.vector.scalar_tensor_tensor(
                    kvb[:], own[:, 8:16], T * EPS, bv8_s[:], OP.add, OP.mult)
                kvc = p_st.tile([128, OTC * B], F32, tag="kvc")
                nc.vector.scalar_tensor_tensor(
                    kvc[:], own[:, 0:8], 1.0 / WSC, kvb[:], OP.mult, OP.add)

                # ---- combine stats for own 2 heads -------------------------
                hs = p_tn.tile([1, OTC * B], F32, tag="tn", name="hs")
                nc.tensor.matmul(hs[:], onc_s[:], own[:, 8:16],
                                 start=True, stop=True)
                den = p_c1.tile([1, OTC * B], F32, tag="den")
                nc.vector.tensor_scalar(den[:], hs[:], EPS * T * HD + EPS,
                                        None, OP.add)
                rden = p_c1.tile([1, OTC * B], F32, tag="rden")
                nc.vector.reciprocal(rden[:], den[:])
                # al carries alpha*OSC (host-folded opre fp8 pre-scale)
                rr = p_c1.tile([1, OTC * B], F32, tag="rr")
                nc.vector.tensor_tensor(rr[:], rden[:], al_s[:], OP.mult)
                bcr = p_tn.tile([128, OTC * B], F32, tag="tn", name="bcr")
                nc.tensor.matmul(bcr[:], onr_s[:], rr[:], start=True,
                                 stop=True)
                kvr = p_c1.tile([128, OTC * B], F32, tag="kvr")
                nc.vector.tensor_tensor(kvr[:], kvc[:], bcr[:], OP.mult)

                # ---- own-head epilogue + row-split Wo ----------------------
                op_ps = p_op.tile([B, D], F32, tag="opp")
                for ol in range(OTC):
                    qkt = p_qk.tile([128, B], F32, tag="qkt")
                    nc.scalar.activation(qkt[:], qps[ol][:], AF.Relu,
                                         bias=bq_s[:, ol:ol + 1],
                                         scale=1.0 / WSC)
                    opre = p_qk.tile([128, B], F8, tag="opre")
                    nc.vector.scalar_tensor_tensor(
                        opre[:], qkt[:], EPS,
                        kvr[:, ol * B:(ol + 1) * B], OP.add, OP.mult)
                    wo_s = wo_sl[ol]
                    for hh in range(4):
                        nc.tensor.matmul(
                            op_ps[:, hh * 512:(hh + 1) * 512], opre[:],
                            wo_s[:, hh * 512:(hh + 1) * 512],
                            start=(ol == 0), stop=(ol == OTC - 1))

                # un-scale (opre*OSC @ wo*WSC), fold in bo/8 per core
                opart = p_c1.tile([B, D], F32, tag="opart")
                nc.vector.scalar_tensor_tensor(
                    opart[:], op_ps[:], 1.0 / (OSC * WSC), bo8_s[:],
                    OP.mult, OP.add)

            # ---- all-reduce partial outputs -------------------------------
            or_in = p_dr.tile([B, D], F32, tag="orin")
            or_out = p_dr.tile([128, (B * D) // 128], F32, tag="orout",
                               addr_space="Shared")
            nc.gpsimd.dma_start(out=or_in[:], in_=opart[:])
            nc.gpsimd.collective_compute(
                "AllReduce", OP.add,
                replica_groups=[list(range(NCORES))],
                ins=[or_in.opt()], outs=[or_out.opt()])
            osum = p_c1.tile([128, (B * D) // 128], F32, tag="osum")
            nc.gpsimd.dma_start(out=osum[:], in_=or_out[:])
            nc.sync.dma_start(out=out_d[:, :], in_=osum[:])

    nc.finalize()
    from concourse import bass_interp
    nc.m = bass_interp.get_hw_module(nc.m)
    return nc


def prep_inputs(q, k_history, v_history, Wq, bq, Wk, bk, Wv, bv, Wo, bo, alpha):
    """Host-side sharding + layout transforms. Returns in_maps for 8 cores."""
    f32 = np.float32

    def wblocks(W):  # [o,d] -> [ot, p(d%128), (d//128)*128 + o_in] f32
        a = W.astype(f32).reshape(NT, 128, NT, 128)       # (ot, o_in, t, p)
        return np.ascontiguousarray(a.transpose(0, 3, 2, 1)) \
                 .reshape(NT, 128, D)

    wkb = wblocks(Wk)
    wvb = wblocks(Wv)
    F8H = ml_dtypes.float8_e4m3
    wqb = wblocks(Wq)
    wob = np.ascontiguousarray(
        Wo.astype(f32).T.reshape(NT, 128, D))               # [ot, p(o_in), o']
    qt = np.ascontiguousarray(
        q.astype(f32).T.reshape(NT, 128, B).transpose(1, 0, 2)
    ).reshape(128, NT * B)                                  # [p, t*4+b]
    bk_t = np.ascontiguousarray(bk.astype(f32).reshape(NT, 128).T)
    bv_t = np.ascontiguousarray(bv.astype(f32).reshape(NT, 128).T)  # [128, NT]
    bq_t = np.ascontiguousarray(bq.astype(f32).reshape(NT, 128).T)
    bo8_r = np.ascontiguousarray(
        np.tile(bo.astype(f32)[None, :] / NCORES, (B, 1)))
    eye = np.eye(128, dtype=f32)
    onc = np.ones((128, 1), f32)
    onr = np.ones((1, 128), f32)
    alpha = np.asarray(alpha, f32)

    qt = qt.astype(F8H)
    shared = dict(qt=qt, bk=bk_t, bo8=bo8_r, eye=eye, onc=onc,
                  onr=onr)

    # cast histories to fp8 once, then per-core strided transpose (1-byte)
    kb = np.asarray(k_history, f32).astype(F8H)             # [T, B, D]
    vb = np.asarray(v_history, f32).astype(F8H)

    in_maps = []
    for c in range(NCORES):
        xk = np.ascontiguousarray(
            kb[c * TLOC:(c + 1) * TLOC].transpose(2, 1, 0)).reshape(D, R)
        xv = np.ascontiguousarray(
            vb[c * TLOC:(c + 1) * TLOC].transpose(2, 1, 0)).reshape(D, R)
        wkv = np.concatenate([wkb[OTC * c:OTC * (c + 1)].reshape(OTC * 128, D),
                              wvb[OTC * c:OTC * (c + 1)].reshape(OTC * 128, D)],
                             axis=0) * WSC
        wkv = wkv.astype(F8H)
        in_maps.append(dict(
            xk=xk, xv=xv, wkv=np.ascontiguousarray(wkv),
            wq=(np.ascontiguousarray(wqb[OTC * c:OTC * (c + 1)])
                .astype(f32) * WSC).astype(F8H),
            wo=(np.ascontiguousarray(wob[OTC * c:OTC * (c + 1)])
                * WSC).astype(F8H),
            bq=np.ascontiguousarray(bq_t[:, OTC * c:OTC * (c + 1)]),
            al=np.ascontiguousarray(
                np.repeat(alpha[OTC * c:OTC * (c + 1)], B)[None, :] * OSC),
            bv8=np.ascontiguousarray(
                np.repeat(bv_t[:, OTC * c:OTC * (c + 1)], B, axis=1)),
            **shared))
    return in_maps


_CACHE = {}


def kernel(**inputs):
    if "nc" not in _CACHE:
        _CACHE["nc"] = build_nc()
    nc = _CACHE["nc"]
    in_maps = prep_inputs(**{k: np.asarray(v) for k, v in inputs.items()})
    res = run_bass_kernel_spmd(nc, in_maps, core_ids=list(range(NCORES)))
    return np.asarray(res.results[0]["out"], dtype=np.float32).reshape(B, D)
